# revision 1
# baseline (speedup 1.0000x reference)
"""GCNConv Trainium2 kernel.

Problem (hardcoded): X [128, 512, 640] f32 packs [A (512) | feat (128)] per
row; W [128, 128] f32.  Output [128, 512, 640] = concat([A, relu(A_norm @
feat @ W)], -1) with A_norm = D^-1/2 (A+I) D^-1/2, deg = rowsum(A).

Algebra used: A_norm @ feat = dis ⊙ ((A+I) @ (dis ⊙ feat)) with
dis = 1/sqrt(deg) — the 512x512 scaled matrix is never materialized, and the
row-side dis factors out of the second matmul: out = relu(dis ⊙ (q @ W)).

Sharding: data-parallel over batch. 8 cores x 16 graphs, W replicated.
"""

from contextlib import ExitStack

import numpy as np

B, N, C_IN, C_OUT = 128, 512, 128, 128
ROW = N + C_IN  # 640
N_CORES = 8
B_LOC = B // N_CORES  # 16
P = 128
NT = N // P  # 4 node tiles per graph

_cache = {}


def _build(n_graphs=B_LOC, repeat=1, bufs=None, eng=None):
    import concourse.mybir as mybir
    import concourse.tile as tile
    from concourse import bacc
    from concourse.masks import make_identity

    f32 = mybir.dt.float32
    nc = bacc.Bacc("TRN2", target_bir_lowering=False, debug=False)

    x_in = nc.declare_dram_parameter("X", [n_graphs, N, ROW], f32, isOutput=False)
    w_in = nc.declare_dram_parameter("W", [C_IN, C_OUT], f32, isOutput=False)
    y_out = nc.declare_dram_parameter("Y", [n_graphs, N, ROW], f32, isOutput=True)

    bf = dict(x=6, featp=2, at=4, ht=2, stat=2, atp=4, p1=2, p2=2)
    if bufs:
        bf.update(bufs)
    # engine choices: "v" = vector/DVE, "a" = scalar/ACT
    en = dict(at="nnnn", ht="n", relu="v", deg="pool", featp="dve", diag="pool")
    if eng:
        en.update(eng)

    with tile.TileContext(nc) as tc, ExitStack() as ctx:
        consts = ctx.enter_context(tc.sbuf_pool(name="consts", bufs=1))
        x_pool = ctx.enter_context(tc.sbuf_pool(name="x", bufs=bf["x"]))
        featp_pool = ctx.enter_context(tc.sbuf_pool(name="featp", bufs=bf["featp"]))
        at_pool = ctx.enter_context(tc.sbuf_pool(name="at", bufs=bf["at"]))
        ht_pool = ctx.enter_context(tc.sbuf_pool(name="ht", bufs=bf["ht"]))
        stat_pool = ctx.enter_context(tc.sbuf_pool(name="stat", bufs=bf["stat"]))
        scr_pool = ctx.enter_context(tc.sbuf_pool(name="scr", bufs=2))
        atp_pool = ctx.enter_context(tc.psum_pool(name="atp", bufs=bf["atp"]))
        p1_pool = ctx.enter_context(tc.psum_pool(name="p1", bufs=bf["p1"]))
        p2_pool = ctx.enter_context(tc.psum_pool(name="p2", bufs=bf["p2"]))

        ident = consts.tile([P, P], f32)
        make_identity(nc, ident)
        w_sb = consts.tile([C_IN, C_OUT], f32)
        nc.sync.dma_start(out=w_sb, in_=w_in[:, :])

        for b in [g for _ in range(repeat) for g in range(n_graphs)]:
            # [512, 640] -> [128 partitions, 4 blocks, 640]; block t holds
            # nodes t*128..t*128+127
            x_dram = x_in[b].rearrange("(t p) c -> p t c", p=P)
            y_dram = y_out[b].rearrange("(t p) c -> p t c", p=P)

            xt = x_pool.tile([P, NT, ROW], f32)
            nc.sync.dma_start(out=xt, in_=x_dram)

            # deg[p, t] = rowsum of A for node t*128+p (A only, no +I)
            deg = stat_pool.tile([P, NT], f32, tag="deg")
            if en["deg"] == "pool":
                # pairwise partial sums on the idle GpSimd engine, final
                # 128-wide reduce on DVE
                s1 = scr_pool.tile([P, NT, N // 2], f32, tag="s1")
                nc.gpsimd.tensor_add(s1, xt[:, :, 0 : N // 2], xt[:, :, N // 2 : N])
                s2 = scr_pool.tile([P, NT, N // 4], f32, tag="s2")
                nc.gpsimd.tensor_add(s2, s1[:, :, 0 : N // 4], s1[:, :, N // 4 :])
                nc.vector.reduce_sum(deg, s2, axis=mybir.AxisListType.X)
            else:
                nc.vector.reduce_sum(deg, xt[:, :, 0:N], axis=mybir.AxisListType.X)
            # dis = where(deg > 0, 1/sqrt(deg), 0) with no inf intermediate:
            # clamp deg to 1 where it is 0, then zero the result via the mask
            mask = stat_pool.tile([P, NT], f32, tag="mask")
            nc.vector.tensor_scalar(
                mask, deg, 0.0, None, op0=mybir.AluOpType.is_gt
            )
            degc = stat_pool.tile([P, NT], f32, tag="degc")
            # degc = deg + (1 - mask)
            nc.vector.tensor_scalar(
                degc, mask, -1.0, 1.0,
                op0=mybir.AluOpType.mult, op1=mybir.AluOpType.add,
            )
            nc.vector.tensor_add(degc, degc, deg)
            sdeg = stat_pool.tile([P, NT], f32, tag="sdeg")
            nc.scalar.sqrt(sdeg, degc)
            rdis = stat_pool.tile([P, NT], f32, tag="rdis")
            nc.vector.reciprocal(rdis, sdeg)
            dis = stat_pool.tile([P, NT], f32, tag="dis")
            nc.vector.tensor_mul(dis, rdis, mask)

            # feat' = dis ⊙ feat ; block t at columns t*128
            featp = featp_pool.tile([P, NT * P], f32)
            for t in range(NT):
                feng = nc.gpsimd if en.get("featp", "pool") == "pool" else nc.vector
                feng.tensor_scalar_mul(
                    featp[:, t * P : (t + 1) * P],
                    xt[:, t, N:ROW],
                    dis[:, t : t + 1],
                )

            # q.T = feat'.T @ (A+I).T accumulated over the 4 m-tiles
            p1 = p1_pool.tile([P, N], f32)
            for km in range(NT):
                atp = atp_pool.tile([P, N], f32)
                for t in range(NT):
                    nc.tensor.transpose(
                        atp[:, t * P : (t + 1) * P],
                        xt[:, t, km * P : (km + 1) * P],
                        ident,
                    )
                at = at_pool.tile([P, N], f32)
                if en["at"][km] == "v":
                    nc.vector.tensor_copy(at, atp)
                elif en["at"][km] == "a":
                    nc.scalar.copy(at, atp)
                else:
                    nc.any.tensor_copy(at, atp)
                # A_hat.T = A.T + I on the diagonal block
                deng = nc.gpsimd if en.get("diag", "pool") == "pool" else nc.vector
                deng.tensor_add(
                    at[:, km * P : (km + 1) * P],
                    at[:, km * P : (km + 1) * P],
                    ident,
                )
                nc.tensor.matmul(
                    p1,
                    featp[:, km * P : (km + 1) * P],
                    at,
                    start=(km == 0),
                    stop=(km == NT - 1),
                )

            ht = ht_pool.tile([P, N], f32)
            if en["ht"] == "v":
                nc.vector.tensor_copy(ht, p1)
            elif en["ht"] == "a":
                nc.scalar.copy(ht, p1)
            else:
                nc.any.tensor_copy(ht, p1)

            # out block t = relu(dis_t ⊙ (q.T[:, t].T @ W)), written into the
            # feat columns of the X tile, then one DMA stores the whole row
            # block (A passes through unchanged).
            for t in range(NT):
                p2 = p2_pool.tile([P, C_OUT], f32)
                nc.tensor.matmul(
                    p2, ht[:, t * P : (t + 1) * P], w_sb, start=True, stop=True
                )
                if en["relu"] == "v":
                    # out = max(p2 * dis, 0) in one DVE tensor-scalar op
                    nc.vector.tensor_scalar(
                        xt[:, t, N:ROW],
                        p2,
                        dis[:, t : t + 1],
                        0.0,
                        op0=mybir.AluOpType.mult,
                        op1=mybir.AluOpType.max,
                    )
                else:
                    nc.scalar.activation(
                        xt[:, t, N:ROW],
                        p2,
                        mybir.ActivationFunctionType.Relu,
                        scale=dis[:, t : t + 1],
                    )

            # store on the ACT HWDGE ring so loads (SP ring) and stores
            # stream in parallel
            nc.scalar.dma_start(out=y_dram, in_=xt)

    nc.finalize()
    return nc


def run(X, W, nc=None):
    from concourse.bass_utils import run_bass_kernel_spmd

    X = np.ascontiguousarray(X, dtype=np.float32)
    W = np.ascontiguousarray(W, dtype=np.float32)
    assert X.shape == (B, N, ROW) and W.shape == (C_IN, C_OUT)

    if nc is None:
        if "nc" not in _cache:
            _cache["nc"] = _build()
        nc = _cache["nc"]

    in_maps = [
        {"X": X[c * B_LOC : (c + 1) * B_LOC], "W": W} for c in range(N_CORES)
    ]
    res = run_bass_kernel_spmd(nc, in_maps, list(range(N_CORES)))
    out = np.concatenate([res.results[c]["Y"] for c in range(N_CORES)], axis=0)
    return out, res


def kernel(X, W):
    out, _ = run(X, W)
    return out



# revision 5
# speedup vs baseline: 32.4993x; 32.4993x over previous
"""GCNConv Trainium2 kernel.

Problem (hardcoded): X [128, 512, 640] f32 packs [A (512) | feat (128)] per
row; W [128, 128] f32.  Output [128, 512, 640] = concat([A, relu(A_norm @
feat @ W)], -1) with A_norm = D^-1/2 (A+I) D^-1/2, deg = rowsum(A).

Algebra used: A_norm @ feat = dis ⊙ ((A+I) @ (dis ⊙ feat)) with
dis = 1/sqrt(deg) — the 512x512 scaled matrix is never materialized, and the
row-side dis factors out of the second matmul: out = relu(dis ⊙ (q @ W)).

Sharding: data-parallel over batch. 8 cores x 16 graphs, W replicated.

Perf notes: the A-block transposes and the (A+I).T accumulation matmuls run
as float32r (1 cycle/row at N=512 vs 4 for fp32); the small q @ W matmuls
run in bf16 with the cast folded into the PSUM->SBUF copy of q. Relative
error stays ~1e-3, well inside the 2e-2 gate.
"""

from contextlib import ExitStack

import numpy as np

B, N, C_IN, C_OUT = 128, 512, 128, 128
ROW = N + C_IN  # 640
N_CORES = 8
B_LOC = B // N_CORES  # 16
P = 128
NT = N // P  # 4 node tiles per graph

_cache = {}


def _build(n_graphs=B_LOC, repeat=1, opts=None):
    import concourse.mybir as mybir
    import concourse.tile as tile
    from concourse import bacc
    from concourse.masks import make_identity

    f32 = mybir.dt.float32
    f32r = mybir.dt.float32r
    bf16 = mybir.dt.bfloat16

    # o["mm"]: dtype of the accumulation matmul operands ("f32r"|"f32").
    #   f32r runs 1 cycle/row at N=512 (vs 4 for f32); the verifier wants
    #   f32r operands *produced* rounded, so featp/at tiles carry the dtype
    #   and their producers (scale op, PSUM->SBUF copy) round for free.
    # o["p2"]: second matmul dtype ("bf16"|"f32")
    # o["deg"]: rowsum strategy ("pool" = gpsimd partial sums, "dve")
    o = dict(mm="f32r", p2="bf16", deg="pool")
    if opts:
        o.update(opts)
    mm_dt = f32r if o["mm"] == "f32r" else f32

    nc = bacc.Bacc("TRN2", target_bir_lowering=False, debug=False)

    x_in = nc.declare_dram_parameter("X", [n_graphs, N, ROW], f32, isOutput=False)
    w_in = nc.declare_dram_parameter("W", [C_IN, C_OUT], f32, isOutput=False)
    y_out = nc.declare_dram_parameter("Y", [n_graphs, N, ROW], f32, isOutput=True)

    with tile.TileContext(nc) as tc, ExitStack() as ctx:
        consts = ctx.enter_context(tc.sbuf_pool(name="consts", bufs=1))
        x_pool = ctx.enter_context(tc.sbuf_pool(name="x", bufs=6))
        featp_pool = ctx.enter_context(tc.sbuf_pool(name="featp", bufs=2))
        at_pool = ctx.enter_context(tc.sbuf_pool(name="at", bufs=4))
        ht_pool = ctx.enter_context(tc.sbuf_pool(name="ht", bufs=2))
        stat_pool = ctx.enter_context(tc.sbuf_pool(name="stat", bufs=2))
        scr_pool = ctx.enter_context(tc.sbuf_pool(name="scr", bufs=2))
        atp_pool = ctx.enter_context(tc.psum_pool(name="atp", bufs=4))
        p1_pool = ctx.enter_context(tc.psum_pool(name="p1", bufs=2))
        p2_pool = ctx.enter_context(tc.psum_pool(name="p2", bufs=2))

        ident = consts.tile([P, P], f32)
        make_identity(nc, ident)
        w_sb = consts.tile([C_IN, C_OUT], f32)
        nc.sync.dma_start(out=w_sb, in_=w_in[:, :])
        if o["p2"] == "bf16":
            w_mm = consts.tile([C_IN, C_OUT], bf16)
            nc.vector.tensor_copy(w_mm, w_sb)
        else:
            w_mm = w_sb

        for b in [g for _ in range(repeat) for g in range(n_graphs)]:
            # [512, 640] -> [128 partitions, 4 blocks, 640]; block t holds
            # nodes t*128..t*128+127
            x_dram = x_in[b].rearrange("(t p) c -> p t c", p=P)
            y_dram = y_out[b].rearrange("(t p) c -> p t c", p=P)

            xt = x_pool.tile([P, NT, ROW], f32)
            nc.sync.dma_start(out=xt, in_=x_dram)

            # deg[p, t] = rowsum of A for node t*128+p (A only, no +I)
            deg = stat_pool.tile([P, NT], f32, tag="deg")
            if o["deg"] == "pool":
                # pairwise partial sums on the idle GpSimd engine, final
                # 128-wide reduce on DVE
                s1 = scr_pool.tile([P, NT, N // 2], f32, tag="s1")
                nc.gpsimd.tensor_add(s1, xt[:, :, 0 : N // 2], xt[:, :, N // 2 : N])
                s2 = scr_pool.tile([P, NT, N // 4], f32, tag="s2")
                nc.gpsimd.tensor_add(s2, s1[:, :, 0 : N // 4], s1[:, :, N // 4 :])
                nc.vector.reduce_sum(deg, s2, axis=mybir.AxisListType.X)
            else:
                nc.vector.reduce_sum(deg, xt[:, :, 0:N], axis=mybir.AxisListType.X)
            # dis = where(deg > 0, 1/sqrt(deg), 0) with no inf intermediate:
            # clamp deg to 1 where it is 0, then zero the result via the mask
            mask = stat_pool.tile([P, NT], f32, tag="mask")
            nc.vector.tensor_scalar(
                mask, deg, 0.0, None, op0=mybir.AluOpType.is_gt
            )
            degc = stat_pool.tile([P, NT], f32, tag="degc")
            # degc = deg + (1 - mask)
            nc.vector.tensor_scalar(
                degc, mask, -1.0, 1.0,
                op0=mybir.AluOpType.mult, op1=mybir.AluOpType.add,
            )
            nc.vector.tensor_add(degc, degc, deg)
            sdeg = stat_pool.tile([P, NT], f32, tag="sdeg")
            nc.scalar.sqrt(sdeg, degc)
            rdis = stat_pool.tile([P, NT], f32, tag="rdis")
            nc.vector.reciprocal(rdis, sdeg)
            dis = stat_pool.tile([P, NT], f32, tag="dis")
            nc.vector.tensor_mul(dis, rdis, mask)

            # feat' = dis ⊙ feat ; block t at columns t*128
            featp = featp_pool.tile([P, NT * P], mm_dt)
            for t in range(NT):
                nc.vector.tensor_scalar_mul(
                    featp[:, t * P : (t + 1) * P],
                    xt[:, t, N:ROW],
                    dis[:, t : t + 1],
                )

            # q.T = feat'.T @ (A+I).T accumulated over the 4 m-tiles
            p1 = p1_pool.tile([P, N], f32)
            for km in range(NT):
                atp = atp_pool.tile([P, N], f32)
                for t in range(NT):
                    nc.tensor.transpose(
                        atp[:, t * P : (t + 1) * P],
                        xt[:, t, km * P : (km + 1) * P],
                        ident,
                    )
                # PSUM -> SBUF copy with A_hat.T = A.T + I merged in: the
                # diagonal 128-block goes through a tensor_add with ident
                at = at_pool.tile([P, N], mm_dt)
                if km > 0:
                    nc.any.tensor_copy(at[:, 0 : km * P], atp[:, 0 : km * P])
                nc.any.tensor_add(
                    at[:, km * P : (km + 1) * P],
                    atp[:, km * P : (km + 1) * P],
                    ident,
                )
                if km < NT - 1:
                    nc.any.tensor_copy(
                        at[:, (km + 1) * P : N], atp[:, (km + 1) * P : N]
                    )
                nc.tensor.matmul(
                    p1,
                    featp[:, km * P : (km + 1) * P],
                    at,
                    start=(km == 0),
                    stop=(km == NT - 1),
                )

            ht = ht_pool.tile([P, N], bf16 if o["p2"] == "bf16" else f32)
            nc.any.tensor_copy(ht, p1)

            # out block t = relu(dis_t ⊙ (q.T[:, t].T @ W)), written into the
            # feat columns of the X tile, then one DMA stores the whole row
            # block (A passes through unchanged).
            for t in range(NT):
                p2 = p2_pool.tile([P, C_OUT], f32)
                nc.tensor.matmul(
                    p2, ht[:, t * P : (t + 1) * P], w_mm, start=True, stop=True
                )
                # out = max(p2 * dis, 0) in one DVE tensor-scalar op
                nc.vector.tensor_scalar(
                    xt[:, t, N:ROW],
                    p2,
                    dis[:, t : t + 1],
                    0.0,
                    op0=mybir.AluOpType.mult,
                    op1=mybir.AluOpType.max,
                )

            # store on the ACT HWDGE ring so loads (SP ring) and stores
            # stream in parallel
            nc.scalar.dma_start(out=y_dram, in_=xt)

    nc.finalize()
    return nc


def run(X, W, nc=None):
    from concourse.bass_utils import run_bass_kernel_spmd

    X = np.ascontiguousarray(X, dtype=np.float32)
    W = np.ascontiguousarray(W, dtype=np.float32)
    assert X.shape == (B, N, ROW) and W.shape == (C_IN, C_OUT)

    if nc is None:
        if "nc" not in _cache:
            _cache["nc"] = _build()
        nc = _cache["nc"]

    in_maps = [
        {"X": X[c * B_LOC : (c + 1) * B_LOC], "W": W} for c in range(N_CORES)
    ]
    res = run_bass_kernel_spmd(nc, in_maps, list(range(N_CORES)))
    out = np.concatenate([res.results[c]["Y"] for c in range(N_CORES)], axis=0)
    return out, res


def kernel(X, W):
    out, _ = run(X, W)
    return out


# revision 12
# speedup vs baseline: 351.7500x; 10.8233x over previous
"""GCNConv Trainium2 kernel.

Problem (hardcoded): X [128, 512, 640] f32 packs [A (512) | feat (128)] per
row; W [128, 128] f32.  Output [128, 512, 640] = concat([A, relu(A_norm @
feat @ W)], -1) with A_norm = D^-1/2 (A+I) D^-1/2, deg = rowsum(A).

Algebra used: A_norm @ feat = dis ⊙ ((A+I) @ (dis ⊙ feat)) with
dis = 1/sqrt(deg) — the 512x512 scaled matrix is never materialized, and the
row-side dis factors out of the second matmul: out = relu(dis ⊙ (q @ W)).

Sharding: data-parallel over batch. 8 cores x 16 graphs, W replicated.

Perf notes: the A-block transposes and the (A+I).T accumulation matmuls run
as float32r (1 cycle/row at N=512 vs 4 for fp32); the small q @ W matmuls
run in bf16 with the cast folded into the PSUM->SBUF copy of q. Relative
error stays ~1e-3, well inside the 2e-2 gate.
"""

from contextlib import ExitStack

import numpy as np

B, N, C_IN, C_OUT = 128, 512, 128, 128
ROW = N + C_IN  # 640
N_CORES = 8
B_LOC = B // N_CORES  # 16
P = 128
NT = N // P  # 4 node tiles per graph

_cache = {}


def _build(n_graphs=B_LOC, repeat=1, opts=None):
    import concourse.mybir as mybir
    import concourse.tile as tile
    from concourse import bacc
    from concourse.masks import make_identity

    f32 = mybir.dt.float32
    f32r = mybir.dt.float32r
    bf16 = mybir.dt.bfloat16

    # o["mm"]: dtype of the accumulation matmul operands ("f32r"|"f32").
    #   f32r runs 1 cycle/row at N=512 (vs 4 for f32); the verifier wants
    #   f32r operands *produced* rounded, so featp/at tiles carry the dtype
    #   and their producers (scale op, PSUM->SBUF copy) round for free.
    # o["p2"]: second matmul dtype ("bf16"|"f32")
    # o["deg"]: rowsum strategy ("pool" = gpsimd partial sums, "dve")
    # o["batch"]: graphs per DMA / stats batch (1 or 2)
    o = dict(
        mm="f32r", p2="bf16", deg="dve", batch=1,
        xb=10, atb=6, fpb=3, htb=2, relu="v",
    )
    if opts:
        o.update(opts)
    mm_dt = f32r if o["mm"] == "f32r" else f32
    G = o["batch"]
    assert n_graphs % G == 0

    nc = bacc.Bacc("TRN2", target_bir_lowering=False, debug=False)

    x_in = nc.declare_dram_parameter("X", [n_graphs, N, ROW], f32, isOutput=False)
    w_in = nc.declare_dram_parameter("W", [C_IN, C_OUT], f32, isOutput=False)
    y_out = nc.declare_dram_parameter("Y", [n_graphs, N, ROW], f32, isOutput=True)

    with tile.TileContext(nc) as tc, ExitStack() as ctx:
        consts = ctx.enter_context(tc.sbuf_pool(name="consts", bufs=1))
        x_pool = ctx.enter_context(
            tc.sbuf_pool(name="x", bufs=o["xb"] or 8 // G)
        )
        featp_pool = ctx.enter_context(tc.sbuf_pool(name="featp", bufs=o["fpb"]))
        at_pool = ctx.enter_context(tc.sbuf_pool(name="at", bufs=o["atb"]))
        ht_pool = ctx.enter_context(tc.sbuf_pool(name="ht", bufs=o["htb"]))
        stat_pool = ctx.enter_context(tc.sbuf_pool(name="stat", bufs=2))
        scr_pool = ctx.enter_context(tc.sbuf_pool(name="scr", bufs=2))
        atp_pool = ctx.enter_context(tc.psum_pool(name="atp", bufs=4))
        p1_pool = ctx.enter_context(tc.psum_pool(name="p1", bufs=2))
        p2_pool = ctx.enter_context(tc.psum_pool(name="p2", bufs=2))

        ident = consts.tile([P, P], f32)
        make_identity(nc, ident)
        w_sb = consts.tile([C_IN, C_OUT], f32)
        nc.sync.dma_start(out=w_sb, in_=w_in[:, :])
        if o["p2"] == "bf16":
            w_mm = consts.tile([C_IN, C_OUT], bf16)
            nc.vector.tensor_copy(w_mm, w_sb)
        else:
            w_mm = w_sb

        for b0 in [
            gg * G for _ in range(repeat) for gg in range(n_graphs // G)
        ]:
            # [G, 512, 640] -> [128 partitions, G graphs, 4 blocks, 640];
            # block t holds nodes t*128..t*128+127
            x_dram = x_in[b0 : b0 + G].rearrange("g (t p) c -> p g t c", p=P)
            y_dram = y_out[b0 : b0 + G].rearrange("g (t p) c -> p g t c", p=P)

            xt = x_pool.tile([P, G, NT, ROW], f32)
            nc.sync.dma_start(out=xt, in_=x_dram)

            # deg[p, g, t] = rowsum of A for node t*128+p (A only, no +I);
            # stats batched over the G graphs of the load
            deg = stat_pool.tile([P, G, NT], f32, tag="deg")
            if o["deg"] == "pool":
                # pairwise partial sums on the idle GpSimd engine, final
                # 128-wide reduce on DVE
                s1 = scr_pool.tile([P, G, NT, N // 2], f32, tag="s1")
                nc.gpsimd.tensor_add(
                    s1, xt[:, :, :, 0 : N // 2], xt[:, :, :, N // 2 : N]
                )
                s2 = scr_pool.tile([P, G, NT, N // 4], f32, tag="s2")
                nc.gpsimd.tensor_add(
                    s2, s1[:, :, :, 0 : N // 4], s1[:, :, :, N // 4 :]
                )
                nc.vector.reduce_sum(deg, s2, axis=mybir.AxisListType.X)
            else:
                nc.vector.reduce_sum(
                    deg, xt[:, :, :, 0:N], axis=mybir.AxisListType.X
                )
            # dis = where(deg > 0, 1/sqrt(deg), 0) with no inf intermediate:
            # clamp deg to 1 where it is 0, then zero the result via the mask
            mask = stat_pool.tile([P, G, NT], f32, tag="mask")
            nc.vector.tensor_scalar(
                mask, deg, 0.0, None, op0=mybir.AluOpType.is_gt
            )
            degc = stat_pool.tile([P, G, NT], f32, tag="degc")
            # degc = deg + (1 - mask)
            nc.vector.tensor_scalar(
                degc, mask, -1.0, 1.0,
                op0=mybir.AluOpType.mult, op1=mybir.AluOpType.add,
            )
            nc.vector.tensor_add(degc, degc, deg)
            sdeg = stat_pool.tile([P, G, NT], f32, tag="sdeg")
            nc.scalar.sqrt(sdeg, degc)
            rdis = stat_pool.tile([P, G, NT], f32, tag="rdis")
            nc.vector.reciprocal(rdis, sdeg)
            dis = stat_pool.tile([P, G, NT], f32, tag="dis")
            nc.vector.tensor_mul(dis, rdis, mask)

            for g in range(G):
                # feat' = dis ⊙ feat ; block t at columns t*128
                featp = featp_pool.tile([P, NT * P], mm_dt)
                for t in range(NT):
                    nc.vector.tensor_scalar_mul(
                        featp[:, t * P : (t + 1) * P],
                        xt[:, g, t, N:ROW],
                        dis[:, g, t : t + 1],
                    )

                # q.T = feat'.T @ (A+I).T accumulated over the 4 m-tiles
                p1 = p1_pool.tile([P, N], f32)
                for km in range(NT):
                    atp = atp_pool.tile([P, N], f32)
                    for t in range(NT):
                        nc.tensor.transpose(
                            atp[:, t * P : (t + 1) * P],
                            xt[:, g, t, km * P : (km + 1) * P],
                            ident,
                        )
                    # PSUM -> SBUF copy with A_hat.T = A.T + I merged in:
                    # the diagonal 128-block goes through a tensor_add
                    at = at_pool.tile([P, N], mm_dt)
                    if km > 0:
                        nc.any.tensor_copy(
                            at[:, 0 : km * P], atp[:, 0 : km * P]
                        )
                    nc.any.tensor_add(
                        at[:, km * P : (km + 1) * P],
                        atp[:, km * P : (km + 1) * P],
                        ident,
                    )
                    if km < NT - 1:
                        nc.any.tensor_copy(
                            at[:, (km + 1) * P : N], atp[:, (km + 1) * P : N]
                        )
                    nc.tensor.matmul(
                        p1,
                        featp[:, km * P : (km + 1) * P],
                        at,
                        start=(km == 0),
                        stop=(km == NT - 1),
                    )

                ht = ht_pool.tile([P, N], bf16 if o["p2"] == "bf16" else f32)
                nc.any.tensor_copy(ht, p1)

                # out block t = relu(dis_t ⊙ (q.T[:, t].T @ W)), written into
                # the feat columns of the X tile, then one DMA stores the
                # whole row block (A passes through unchanged).
                for t in range(NT):
                    p2 = p2_pool.tile([P, C_OUT], f32)
                    nc.tensor.matmul(
                        p2,
                        ht[:, t * P : (t + 1) * P],
                        w_mm,
                        start=True,
                        stop=True,
                    )
                    if o["relu"] == "a":
                        # relu with the dis scale on the ACT engine
                        nc.scalar.activation(
                            xt[:, g, t, N:ROW],
                            p2,
                            mybir.ActivationFunctionType.Relu,
                            scale=dis[:, g, t : t + 1],
                        )
                    else:
                        # out = max(p2 * dis, 0) in one DVE tensor-scalar op
                        nc.vector.tensor_scalar(
                            xt[:, g, t, N:ROW],
                            p2,
                            dis[:, g, t : t + 1],
                            0.0,
                            op0=mybir.AluOpType.mult,
                            op1=mybir.AluOpType.max,
                        )

            # store on the ACT HWDGE ring so loads (SP ring) and stores
            # stream in parallel
            nc.scalar.dma_start(out=y_dram, in_=xt)

    nc.finalize()
    return nc


def run(X, W, nc=None):
    from concourse.bass_utils import run_bass_kernel_spmd

    X = np.ascontiguousarray(X, dtype=np.float32)
    W = np.ascontiguousarray(W, dtype=np.float32)
    assert X.shape == (B, N, ROW) and W.shape == (C_IN, C_OUT)

    if nc is None:
        if "nc" not in _cache:
            _cache["nc"] = _build()
        nc = _cache["nc"]

    in_maps = [
        {"X": X[c * B_LOC : (c + 1) * B_LOC], "W": W} for c in range(N_CORES)
    ]
    res = run_bass_kernel_spmd(nc, in_maps, list(range(N_CORES)))
    out = np.concatenate([res.results[c]["Y"] for c in range(N_CORES)], axis=0)
    return out, res


def kernel(X, W):
    out, _ = run(X, W)
    return out


# revision 16
# speedup vs baseline: 360.7961x; 1.0257x over previous
"""GCNConv Trainium2 kernel.

Problem (hardcoded): X [128, 512, 640] f32 packs [A (512) | feat (128)] per
row; W [128, 128] f32.  Output [128, 512, 640] = concat([A, relu(A_norm @
feat @ W)], -1) with A_norm = D^-1/2 (A+I) D^-1/2, deg = rowsum(A).

Algebra used: A_norm @ feat = dis ⊙ ((A+I) @ (dis ⊙ feat)) with
dis = 1/sqrt(deg) — the 512x512 scaled matrix is never materialized, and the
row-side dis factors out of the second matmul: out = relu(dis ⊙ (q @ W)).

Sharding: data-parallel over batch. 8 cores x 16 graphs, W replicated.

Perf notes: the A-block transposes and the (A+I).T accumulation matmuls run
as float32r (1 cycle/row at N=512 vs 4 for fp32); the small q @ W matmuls
run in bf16 with the cast folded into the PSUM->SBUF copy of q. Relative
error stays ~1e-3, well inside the 2e-2 gate.
"""

from contextlib import ExitStack

import numpy as np

B, N, C_IN, C_OUT = 128, 512, 128, 128
ROW = N + C_IN  # 640
N_CORES = 8
B_LOC = B // N_CORES  # 16
P = 128
NT = N // P  # 4 node tiles per graph

_cache = {}


def _build(n_graphs=B_LOC, repeat=1, opts=None):
    import concourse.mybir as mybir
    import concourse.tile as tile
    from concourse import bacc
    from concourse.masks import make_identity

    f32 = mybir.dt.float32
    f32r = mybir.dt.float32r
    bf16 = mybir.dt.bfloat16

    # o["mm"]: dtype of the accumulation matmul operands ("f32r"|"f32").
    #   f32r runs 1 cycle/row at N=512 (vs 4 for f32); the verifier wants
    #   f32r operands *produced* rounded, so featp/at tiles carry the dtype
    #   and their producers (scale op, PSUM->SBUF copy) round for free.
    # o["p2"]: second matmul dtype ("bf16"|"f32")
    # o["deg"]: rowsum strategy ("pool" = gpsimd partial sums, "dve")
    # o["batch"]: graphs per DMA / stats batch (1 or 2)
    # engine routing: "at" = PSUM->SBUF copy of A.T blocks ("n" any / "a"
    # ACT + Pool diag), "fp" = featp scale ("v" DVE / "p" Pool), "relu"
    # ("v" DVE / "a" ACT), "ht" = q copy ("n" any / "a" ACT)
    o = dict(
        mm="f32r", p2="bf16", deg="dve", batch=1,
        xb=10, atb=6, fpb=3, htb=2, relu="v", at="n", fp="v", ht="n",
    )
    if opts:
        o.update(opts)
    mm_dt = f32r if o["mm"] == "f32r" else f32
    G = o["batch"]
    assert n_graphs % G == 0

    nc = bacc.Bacc("TRN2", target_bir_lowering=False, debug=False)

    x_in = nc.declare_dram_parameter("X", [n_graphs, N, ROW], f32, isOutput=False)
    w_in = nc.declare_dram_parameter("W", [C_IN, C_OUT], f32, isOutput=False)
    y_out = nc.declare_dram_parameter("Y", [n_graphs, N, ROW], f32, isOutput=True)

    with tile.TileContext(nc) as tc, ExitStack() as ctx:
        consts = ctx.enter_context(tc.sbuf_pool(name="consts", bufs=1))
        x_pool = ctx.enter_context(
            tc.sbuf_pool(name="x", bufs=o["xb"] or 8 // G)
        )
        featp_pool = ctx.enter_context(tc.sbuf_pool(name="featp", bufs=o["fpb"]))
        at_pool = ctx.enter_context(tc.sbuf_pool(name="at", bufs=o["atb"]))
        ht_pool = ctx.enter_context(tc.sbuf_pool(name="ht", bufs=o["htb"]))
        stat_pool = ctx.enter_context(tc.sbuf_pool(name="stat", bufs=2))
        scr_pool = ctx.enter_context(tc.sbuf_pool(name="scr", bufs=2))
        atp_pool = ctx.enter_context(tc.psum_pool(name="atp", bufs=4))
        p1_pool = ctx.enter_context(tc.psum_pool(name="p1", bufs=2))
        p2_pool = ctx.enter_context(tc.psum_pool(name="p2", bufs=2))

        ident = consts.tile([P, P], f32)
        make_identity(nc, ident)
        w_sb = consts.tile([C_IN, C_OUT], f32)
        nc.sync.dma_start(out=w_sb, in_=w_in[:, :])
        if o["p2"] == "bf16":
            w_mm = consts.tile([C_IN, C_OUT], bf16)
            nc.vector.tensor_copy(w_mm, w_sb)
        else:
            w_mm = w_sb

        for b0 in [
            gg * G for _ in range(repeat) for gg in range(n_graphs // G)
        ]:
            # [G, 512, 640] -> [128 partitions, G graphs, 4 blocks, 640];
            # block t holds nodes t*128..t*128+127
            x_dram = x_in[b0 : b0 + G].rearrange("g (t p) c -> p g t c", p=P)
            y_dram = y_out[b0 : b0 + G].rearrange("g (t p) c -> p g t c", p=P)

            xt = x_pool.tile([P, G, NT, ROW], f32)
            nc.sync.dma_start(out=xt, in_=x_dram)

            # deg[p, g, t] = rowsum of A for node t*128+p (A only, no +I);
            # stats batched over the G graphs of the load
            deg = stat_pool.tile([P, G, NT], f32, tag="deg")
            if o["deg"] == "pool":
                # pairwise partial sums on the idle GpSimd engine, final
                # 128-wide reduce on DVE
                s1 = scr_pool.tile([P, G, NT, N // 2], f32, tag="s1")
                nc.gpsimd.tensor_add(
                    s1, xt[:, :, :, 0 : N // 2], xt[:, :, :, N // 2 : N]
                )
                s2 = scr_pool.tile([P, G, NT, N // 4], f32, tag="s2")
                nc.gpsimd.tensor_add(
                    s2, s1[:, :, :, 0 : N // 4], s1[:, :, :, N // 4 :]
                )
                nc.vector.reduce_sum(deg, s2, axis=mybir.AxisListType.X)
            else:
                nc.vector.reduce_sum(
                    deg, xt[:, :, :, 0:N], axis=mybir.AxisListType.X
                )
            # dis = where(deg > 0, 1/sqrt(deg), 0) with no inf intermediate:
            # clamp deg to 1 where it is 0, then zero the result via the mask
            mask = stat_pool.tile([P, G, NT], f32, tag="mask")
            nc.vector.tensor_scalar(
                mask, deg, 0.0, None, op0=mybir.AluOpType.is_gt
            )
            degc = stat_pool.tile([P, G, NT], f32, tag="degc")
            # degc = deg + (1 - mask)
            nc.vector.tensor_scalar(
                degc, mask, -1.0, 1.0,
                op0=mybir.AluOpType.mult, op1=mybir.AluOpType.add,
            )
            nc.vector.tensor_add(degc, degc, deg)
            sdeg = stat_pool.tile([P, G, NT], f32, tag="sdeg")
            nc.scalar.sqrt(sdeg, degc)
            rdis = stat_pool.tile([P, G, NT], f32, tag="rdis")
            nc.vector.reciprocal(rdis, sdeg)
            dis = stat_pool.tile([P, G, NT], f32, tag="dis")
            nc.vector.tensor_mul(dis, rdis, mask)

            for g in range(G):
                # feat' = dis ⊙ feat ; block t at columns t*128
                featp = featp_pool.tile([P, NT * P], mm_dt)
                fp_eng = nc.gpsimd if o["fp"] == "p" else nc.vector
                for t in range(NT):
                    fp_eng.tensor_scalar_mul(
                        featp[:, t * P : (t + 1) * P],
                        xt[:, g, t, N:ROW],
                        dis[:, g, t : t + 1],
                    )

                # q.T = feat'.T @ (A+I).T accumulated over the 4 m-tiles
                p1 = p1_pool.tile([P, N], f32)
                for km in range(NT):
                    atp = atp_pool.tile([P, N], f32)
                    for t in range(NT):
                        nc.tensor.transpose(
                            atp[:, t * P : (t + 1) * P],
                            xt[:, g, t, km * P : (km + 1) * P],
                            ident,
                        )
                    # PSUM -> SBUF copy with A_hat.T = A.T + I merged in:
                    # the diagonal 128-block goes through a tensor_add
                    at = at_pool.tile([P, N], mm_dt)
                    if o["at"] == "a":
                        # off-diagonal columns on the ACT engine, +I diag
                        # block on the (otherwise idle) GpSimd engine
                        cp = nc.scalar.copy
                        add = nc.gpsimd.tensor_add
                    else:
                        cp = nc.any.tensor_copy
                        add = nc.any.tensor_add
                    if km > 0:
                        cp(at[:, 0 : km * P], atp[:, 0 : km * P])
                    add(
                        at[:, km * P : (km + 1) * P],
                        atp[:, km * P : (km + 1) * P],
                        ident,
                    )
                    if km < NT - 1:
                        cp(at[:, (km + 1) * P : N], atp[:, (km + 1) * P : N])
                    nc.tensor.matmul(
                        p1,
                        featp[:, km * P : (km + 1) * P],
                        at,
                        start=(km == 0),
                        stop=(km == NT - 1),
                    )

                ht = ht_pool.tile([P, N], bf16 if o["p2"] == "bf16" else f32)
                if o["ht"] == "a":
                    nc.scalar.copy(ht, p1)
                else:
                    nc.any.tensor_copy(ht, p1)

                # out block t = relu(dis_t ⊙ (q.T[:, t].T @ W)), written into
                # the feat columns of the X tile, then one DMA stores the
                # whole row block (A passes through unchanged).
                for t in range(NT):
                    p2 = p2_pool.tile([P, C_OUT], f32)
                    nc.tensor.matmul(
                        p2,
                        ht[:, t * P : (t + 1) * P],
                        w_mm,
                        start=True,
                        stop=True,
                    )
                    if o["relu"] == "a":
                        # relu with the dis scale on the ACT engine
                        nc.scalar.activation(
                            xt[:, g, t, N:ROW],
                            p2,
                            mybir.ActivationFunctionType.Relu,
                            scale=dis[:, g, t : t + 1],
                        )
                    else:
                        # out = max(p2 * dis, 0) in one DVE tensor-scalar op
                        nc.vector.tensor_scalar(
                            xt[:, g, t, N:ROW],
                            p2,
                            dis[:, g, t : t + 1],
                            0.0,
                            op0=mybir.AluOpType.mult,
                            op1=mybir.AluOpType.max,
                        )

            # store on the ACT HWDGE ring so loads (SP ring) and stores
            # stream in parallel
            nc.scalar.dma_start(out=y_dram, in_=xt)

    nc.finalize()
    return nc


def run(X, W, nc=None):
    from concourse.bass_utils import run_bass_kernel_spmd

    X = np.ascontiguousarray(X, dtype=np.float32)
    W = np.ascontiguousarray(W, dtype=np.float32)
    assert X.shape == (B, N, ROW) and W.shape == (C_IN, C_OUT)

    if nc is None:
        if "nc" not in _cache:
            _cache["nc"] = _build()
        nc = _cache["nc"]

    in_maps = [
        {"X": X[c * B_LOC : (c + 1) * B_LOC], "W": W} for c in range(N_CORES)
    ]
    res = run_bass_kernel_spmd(nc, in_maps, list(range(N_CORES)))
    out = np.concatenate([res.results[c]["Y"] for c in range(N_CORES)], axis=0)
    return out, res


def kernel(X, W):
    out, _ = run(X, W)
    return out
